# revision 31
# baseline (speedup 1.0000x reference)
"""ClusterAttention TRN2 kernel (v3).

Computation (per batch item):
  a  = conv1d_same(x, w1, b1)                      # (k, p) conv over p
  bm = conv1d_same(x^T, w2, b2)                    # (k, c) conv over c
  bm = bm @ x                                      # (k, p)
  probs = softmax(a + bm, axis=k)                  # (k, p)
  out = w_post @ (probs[:, None, :] * x[None]).reshape(k*c, p) + b_post

Sharding: data-parallel over batch n (32) across 8 cores, 4 per core.
All params replicated.

v3 design:
  - convs as dense matmuls with taps stacked on output partitions, then a
    log-tree of shifted adds on DVE (a-tree in bf16, bm-tree in f32)
  - logits computed DIRECTLY transposed: pa4T[p, k] per 128-wide p-window
    via selector matmuls (stationary = tree output D, moving = selT) plus
    x^T @ bmT matmuls; softmax runs on [p, k] tiles with no PE transposes
  - softmax normalization batched per half: one reciprocal + wide DVE
    multiplies (free-dim broadcast APs) instead of per-window ops
  - probs -> DRAM via XBAR dma transpose; broadcast back to [128, p] tiles
    with stride-0 partition-broadcast DMA reads (no gpsimd)
  - xT uploaded from host; bias adds on Act; consts loaded in few DMAs
"""
import sys

sys.path.insert(0, "/opt/trn_rl_repo")

import numpy as np

import concourse.bass as bass
import concourse.mybir as mybir
import concourse.tile as tile
from concourse import bacc
from concourse.bass_utils import run_bass_kernel_spmd

dt = mybir.dt

N_CORES = 8
B = 4            # batch items per core
C = 256          # channels
P = 2048         # sequence length
K = 16           # clusters
TAPS = 15
PAD = 7
CT = C // 128    # c-tiles
NCH = P // 512   # 512-wide p-chunks
PW = P // 128    # 128-wide p-windows
HPW = PW // 2    # p-windows per half
KG = 2           # clusters per broadcast group

AW = P + 16      # A_buf width: col j holds A[p = j - PAD], zeros outside
BMW = C + 16     # BM_buf width


def build_nc():
    nc = bacc.Bacc(None)

    xs_d = nc.dram_tensor("xs", [B, C, P], dt.float32, kind="ExternalInput")
    xt_d = nc.dram_tensor("xt", [B, 128, PW * C], dt.float32, kind="ExternalInput")
    w1r_d = nc.dram_tensor("w1r", [C, 256], dt.float32, kind="ExternalInput")
    w2r_d = nc.dram_tensor("w2r", [P, 256], dt.float32, kind="ExternalInput")
    wpt_d = nc.dram_tensor("wpt", [K * C, C], dt.bfloat16, kind="ExternalInput")
    b2_d = nc.dram_tensor("b2c", [K, 1], dt.float32, kind="ExternalInput")
    bp_d = nc.dram_tensor("bpc", [128, CT], dt.float32, kind="ExternalInput")
    eb1_d = nc.dram_tensor("expb1", [128, HPW * K], dt.float32, kind="ExternalInput")
    id_d = nc.dram_tensor("ident", [128, 128], dt.float32, kind="ExternalInput")
    sel_d = nc.dram_tensor("sel", [32, 32], dt.float32, kind="ExternalInput")
    selt_d = nc.dram_tensor("selt", [32, 32], dt.bfloat16, kind="ExternalInput")
    idb_d = nc.dram_tensor("identb", [128, 128], dt.bfloat16, kind="ExternalInput")
    out_d = nc.dram_tensor("out", [B, C, P], dt.float32, kind="ExternalOutput")

    F32R = dt.float32r
    BF16 = dt.bfloat16
    IDENT = mybir.ActivationFunctionType.Identity
    EXP = mybir.ActivationFunctionType.Exp

    with tile.TileContext(nc) as tc:
        with (
            tc.tile_pool(name="const", bufs=1) as cpool,
            tc.tile_pool(name="xs", bufs=2) as xpool,
            tc.tile_pool(name="xt", bufs=2) as xtpool,
            tc.tile_pool(name="abuf", bufs=1) as apool,
            tc.tile_pool(name="tree", bufs=2) as tpool,
            tc.tile_pool(name="sm", bufs=2) as smpool,
            tc.tile_pool(name="main", bufs=4) as mpool,
            tc.tile_pool(name="bc", bufs=4) as bcpool,
            tc.tile_pool(name="oc", bufs=2) as ocpool,
            tc.tile_pool(name="pd", bufs=2, space="DRAM") as pdpool,
            tc.tile_pool(name="pconv", bufs=2, space="PSUM") as pconv,
            tc.tile_pool(name="ptr", bufs=1, space="PSUM") as ptr,
            tc.tile_pool(name="pout", bufs=4, space="PSUM") as pout,
        ):
            # ---- constants (few, consolidated DMAs) ----
            w1rt = cpool.tile([128, 2, 256], F32R, tag="w1rt")
            nc.sync.dma_start(
                out=w1rt[:],
                in_=w1r_d.rearrange("(t i) c -> i t c", t=2).bitcast(F32R))
            ident = cpool.tile([128, 128], dt.float32, tag="ident")
            nc.sync.dma_start(out=ident[:], in_=id_d[:])
            sel = cpool.tile([32, 32], F32R, tag="sel")
            nc.sync.dma_start(out=sel[:], in_=sel_d[:].bitcast(F32R))
            selt = cpool.tile([32, 32], BF16, tag="selt")
            nc.sync.dma_start(out=selt[:], in_=selt_d[:])
            w2rt = cpool.tile([128, PW, 256], F32R, tag="w2rt")
            wptt = cpool.tile([128, 2 * K, 256], BF16, tag="wptt")
            b2c = cpool.tile([K, 1], dt.float32, tag="b2c")
            nc.sync.dma_start(out=b2c[:], in_=b2_d[:])
            bpc = cpool.tile([128, CT], dt.float32, tag="bpc")
            nc.sync.dma_start(out=bpc[:], in_=bp_d[:])
            expb1 = cpool.tile([128, HPW * K], dt.float32, tag="expb1")
            nc.sync.dma_start(out=expb1[:], in_=eb1_d[:])

            state = {}

            def emit_p1(b):
                """loads, xb convert, convs, trees, bm merge, bmT."""
                # ---- load x and xT ----
                xs = [xpool.tile([128, P], F32R, tag=f"xs{ct}", name=f"xs{ct}")
                      for ct in range(CT)]
                for ct in range(CT):
                    for hh in range(2):
                        nc.sync.dma_start(
                            out=xs[ct][:, hh * 1024:(hh + 1) * 1024],
                            in_=xs_d[b, ct * 128:(ct + 1) * 128,
                                     hh * 1024:(hh + 1) * 1024].bitcast(F32R))
                xT = xtpool.tile([128, PW * C], F32R, tag="xT", name="xT")
                if b == 0:
                    # batch 0: build xT on-chip (PE is idle during the
                    # DMA-bound startup; saves 2.2MB off the critical window)
                    for pw in range(PW):
                        for ct in range(CT):
                            ptx = pconv.tile([128, 512], dt.float32,
                                             tag="pconv", name="ptx")
                            nc.tensor.transpose(
                                ptx[0:128, 0:128],
                                xs[ct][:, pw * 128:(pw + 1) * 128]
                                .bitcast(dt.float32),
                                ident[:])
                            nc.scalar.copy(
                                xT[:, pw * C + ct * 128:pw * C + (ct + 1) * 128],
                                ptx[0:128, 0:128])
                    for hh in range(4):
                        nc.sync.dma_start(
                            out=w2rt[:, hh * 4:(hh + 1) * 4, :],
                            in_=w2r_d[hh * 512:(hh + 1) * 512]
                            .rearrange("(t i) c -> i t c", t=4).bitcast(F32R))
                else:
                    for hh in range(4):
                        nc.sync.dma_start(
                            out=xT[:, hh * 1024:(hh + 1) * 1024],
                            in_=xt_d[b, :, hh * 1024:(hh + 1) * 1024]
                            .bitcast(F32R))

                # ---- bf16 copy of x for the y path ----
                xb = [xpool.tile([128, P], BF16, tag=f"xb{ct}",
                                 name=f"xb{ct}", bufs=3) for ct in range(CT)]
                for ct in range(CT):
                    nc.scalar.copy(xb[ct][:], xs[ct][:].bitcast(dt.float32))

                # ---- A conv matmuls: A[(t,kk), p] (bf16 store) ----
                A = [apool.tile([128, AW], BF16, tag=f"A{mc}", name=f"A{mc}")
                     for mc in range(2)]
                for mc in range(2):
                    nc.vector.memset(A[mc][:, 0:PAD], 0.0)
                    nc.vector.memset(A[mc][:, PAD + P:AW], 0.0)
                for mc in range(2):
                    for nch in range(NCH):
                        pa = pconv.tile([128, 512], dt.float32, tag="pconv")
                        for ct in range(CT):
                            nc.tensor.matmul(
                                pa[:],
                                w1rt[:, ct, mc * 128:(mc + 1) * 128],
                                xs[ct][:, nch * 512:(nch + 1) * 512],
                                start=(ct == 0), stop=(ct == CT - 1))
                        nc.scalar.copy(
                            A[mc][:, PAD + nch * 512:PAD + (nch + 1) * 512], pa[:])

                # ---- BM conv matmuls: BM[(t,kk), cc] (f32) ----
                BM = [apool.tile([128, BMW], dt.float32, tag=f"BM{mc}",
                                 name=f"BM{mc}") for mc in range(2)]
                for mc in range(2):
                    nc.vector.memset(BM[mc][:, 0:PAD], 0.0)
                    nc.vector.memset(BM[mc][:, PAD + C:BMW], 0.0)
                for mc in range(2):
                    pb = pconv.tile([128, C], dt.float32, tag="pconv")
                    for pw in range(PW):
                        nc.tensor.matmul(
                            pb[:],
                            w2rt[:, pw, mc * 128:(mc + 1) * 128],
                            xT[:, pw * C:pw * C + C],
                            start=(pw == 0), stop=(pw == PW - 1))
                    nc.scalar.copy(BM[mc][:, PAD:PAD + C], pb[:])

                # ---- shift-add tree for a (bf16) ----
                HALF = P // 2
                Da = []
                for h in range(2):
                    o = h * HALF
                    w1_, w2_, w3_ = HALF + 8, HALF + 4, HALF + 2
                    Bl = tpool.tile([64, w1_], BF16, tag="tr", name="Bl")
                    Bh = tpool.tile([64, w1_], BF16, tag="tr", name="Bh")
                    nc.vector.tensor_add(Bl[:], A[0][0:64, o:o + w1_],
                                         A[1][0:64, o + 8:o + 8 + w1_])
                    nc.vector.tensor_add(Bh[:], A[0][64:128, o:o + w1_],
                                         A[1][64:128, o + 8:o + 8 + w1_])
                    Cl = tpool.tile([32, w2_], BF16, tag="trc", name="Cl")
                    Ch = tpool.tile([32, w2_], BF16, tag="trc", name="Ch")
                    nc.vector.tensor_add(Cl[:], Bl[0:32, 0:w2_],
                                         Bh[0:32, 4:4 + w2_])
                    nc.vector.tensor_add(Ch[:], Bl[32:64, 0:w2_],
                                         Bh[32:64, 4:4 + w2_])
                    D = tpool.tile([32, w3_], BF16, tag="trd", name="D", bufs=2)
                    nc.vector.tensor_add(D[:], Cl[0:32, 0:w3_],
                                         Ch[0:32, 2:2 + w3_])
                    Da.append(D)

                # ---- shift-add tree for bm: [16, C] (f32) ----
                bmt = smpool.tile([K, C], dt.float32, tag="bmt")
                w1_, w2_, w3_ = C + 8, C + 4, C + 2
                Bl = tpool.tile([64, w1_], dt.float32, tag="trb", name="Blb")
                Bh = tpool.tile([64, w1_], dt.float32, tag="trb", name="Bhb")
                nc.vector.tensor_add(Bl[:], BM[0][0:64, 0:w1_],
                                     BM[1][0:64, 8:8 + w1_])
                nc.vector.tensor_add(Bh[:], BM[0][64:128, 0:w1_],
                                     BM[1][64:128, 8:8 + w1_])
                Cl = tpool.tile([32, w2_], dt.float32, tag="trcb", name="Clb")
                Ch = tpool.tile([32, w2_], dt.float32, tag="trcb", name="Chb")
                nc.vector.tensor_add(Cl[:], Bl[0:32, 0:w2_], Bh[0:32, 4:4 + w2_])
                nc.vector.tensor_add(Ch[:], Bl[32:64, 0:w2_], Bh[32:64, 4:4 + w2_])
                Db = tpool.tile([32, w3_], F32R, tag="trdb", name="Db", bufs=2)
                nc.vector.tensor_add(Db[:], Cl[0:32, 0:w3_],
                                     Ch[0:32, 2:2 + w3_])
                pb4 = pconv.tile([K, C], dt.float32, tag="pconv", name="pb4")
                nc.tensor.matmul(pb4[:], sel[:, 0:16], Db[:, 0:C],
                                 start=True, stop=False)
                nc.tensor.matmul(pb4[:], sel[:, 16:32], Db[:, 1:1 + C],
                                 start=False, stop=True)
                nc.scalar.activation(bmt[:], pb4[:], IDENT, bias=b2c[:])

                # ---- bmT = bm^T [C, 16] (2 tiles) ----
                bmT = [smpool.tile([128, K], F32R, tag=f"bmT{ct}", name=f"bmT{ct}")
                       for ct in range(CT)]
                for ct in range(CT):
                    pt = pconv.tile([128, 512], dt.float32, tag="pconv", name="pt")
                    nc.tensor.transpose(
                        pt[0:128, 0:K], bmt[:, ct * 128:(ct + 1) * 128],
                        ident[0:K, 0:K])
                    nc.scalar.copy(bmT[ct][:], pt[0:128, 0:K])
                state[b] = dict(xs=xs, xb=xb, Da=Da, bmT=bmT)

            def emit_p2h(b, h):
                """transposed logits + softmax + probs -> DRAM for one half."""
                xs = state[b]["xs"]
                D = state[b]["Da"][h]
                bmT = state[b]["bmT"]
                if h == 0:
                    state[b]["probs_d"] = pdpool.tile(
                        [K, P], BF16, tag="probs_d", name="probs_d")
                probs_d = state[b]["probs_d"]
                # pa4T[p, k] = a-merge (selector mms on D) + x^T @ bmT
                expTh = smpool.tile([128, HPW * K], dt.float32,
                                    tag="expTh", name="expTh")
                esums = smpool.tile([128, HPW], dt.float32, tag="esums",
                                    name="esums")
                for pl in range(HPW):
                    pw = h * HPW + pl
                    o = pl * 128
                    pa4T = ptr.tile([128, K], dt.float32, tag="pa4T",
                                    bufs=2)
                    nc.tensor.matmul(pa4T[:], D[:, o:o + 128],
                                     selt[:, 0:16], start=True, stop=False)
                    nc.tensor.matmul(pa4T[:], D[:, o + 1:o + 129],
                                     selt[:, 16:32], start=False, stop=False)
                    for ct in range(CT):
                        nc.tensor.matmul(
                            pa4T[:],
                            xs[ct][:, pw * 128:(pw + 1) * 128],
                            bmT[ct][:],
                            start=False, stop=(ct == CT - 1))
                    negmx = smpool.tile([128, 1], dt.float32, tag="negmx")
                    nc.vector.tensor_reduce(
                        negmx[:], pa4T[:], axis=mybir.AxisListType.X,
                        op=mybir.AluOpType.max, negate=True)
                    nc.scalar.activation(
                        expTh[:, pl * K:(pl + 1) * K], pa4T[:],
                        EXP, bias=negmx[:])
                # esums/normalize batched over the half
                tmpE = smpool.tile([128, HPW * K], dt.float32, tag="tmpE",
                                   name="tmpE")
                nc.vector.tensor_mul(tmpE[:], expTh[:], expb1[:])
                nc.vector.tensor_reduce(
                    esums[:], tmpE[:].rearrange("p (a b) -> p a b", a=HPW),
                    axis=mybir.AxisListType.X, op=mybir.AluOpType.add)
                recips = smpool.tile([128, HPW], dt.float32, tag="recips",
                                     name="recips")
                nc.vector.reciprocal(recips[:], esums[:])
                pTh = smpool.tile([128, HPW * K], dt.float32, tag="probsTh",
                                  name="probsTh")
                nc.vector.tensor_mul(
                    pTh[:].rearrange("p (a b) -> p a b", a=HPW),
                    tmpE[:].rearrange("p (a b) -> p a b", a=HPW),
                    recips[:].unsqueeze(2).to_broadcast([128, HPW, K]))
                # probs -> DRAM: ppb[(a k), i] = pTh[i, a*K + k]; the DMA
                # scatters partition a*K+k to probs_d[k, h*1024 + a*128 + i]
                ppb = pconv.tile([128, 512], dt.float32, tag="pconv",
                                 name="ppb")
                nc.tensor.transpose(ppb[0:128, 0:128], pTh[:], ident[:])
                probsS = smpool.tile([128, 128], BF16, tag="probsS",
                                     name="probsS")
                nc.scalar.copy(probsS[:], ppb[0:128, 0:128])
                nc.scalar.dma_start(
                    out=probs_d[:, h * 1024:(h + 1) * 1024]
                    .rearrange("k (a i) -> a k i", a=HPW),
                    in_=probsS[:])
                bcts = []
                for kg in range(K // KG):
                    bct = bcpool.tile([128, KG, 1024], BF16,
                                      tag="bct", name="bct")
                    nc.sync.dma_start(
                        out=bct[:],
                        in_=probs_d[kg * KG:(kg + 1) * KG,
                                    h * 1024:(h + 1) * 1024]
                        .unsqueeze(0).partition_broadcast(128))
                    bcts.append(bct)
                state[b][f"bcts{h}"] = bcts
                if b == 0 and h == 0:
                    for hh in range(4):
                        nc.sync.dma_start(
                            out=wptt[:, hh * 8:(hh + 1) * 8, :],
                            in_=wpt_d[hh * 1024:(hh + 1) * 1024]
                            .rearrange("(t i) c -> i t c", t=8))

            def emit_p3h(b, half):
                """y build, main matmul, bias + store for one half."""
                xb = state[b]["xb"]
                chunks = [2 * half, 2 * half + 1]
                po = {}
                for nch in chunks:
                    for ot in range(CT):
                        po[(nch, ot)] = pout.tile(
                            [128, 512], dt.float32, tag="pout", name="po")
                for kg in range(K // KG):
                    bct = state[b][f"bcts{half}"][kg]
                    for j in range(KG):
                        kk = kg * KG + j
                        ys = []
                        for ct in range(CT):
                            y = mpool.tile([128, 1024], BF16,
                                           tag="y", name="y")
                            nc.vector.tensor_mul(
                                y[:],
                                xb[ct][:, half * 1024:(half + 1) * 1024],
                                bct[:, j, :])
                            ys.append(y)
                        for nch in chunks:
                            co = (nch % 2) * 512
                            for ct in range(CT):
                                for ot in range(CT):
                                    nc.tensor.matmul(
                                        po[(nch, ot)][:],
                                        wptt[:, kk * 2 + ct,
                                             ot * 128:(ot + 1) * 128],
                                        ys[ct][:, co:co + 512],
                                        start=(kg == 0 and j == 0 and ct == 0),
                                        stop=(kg == K // KG - 1 and j == KG - 1
                                              and ct == CT - 1))
                for ot in range(CT):
                    oc = ocpool.tile([128, 1024], dt.float32, tag="oc",
                                     name="oc")
                    for nch in chunks:
                        nc.scalar.activation(
                            oc[:, (nch % 2) * 512:(nch % 2) * 512 + 512],
                            po[(nch, ot)][:], IDENT,
                            bias=bpc[:, ot:ot + 1])
                    nc.scalar.dma_start(
                        out=out_d[b, ot * 128:(ot + 1) * 128,
                                  half * 1024:(half + 1) * 1024],
                        in_=oc[:])

            emit_p1(0)
            emit_p2h(0, 0)
            emit_p2h(0, 1)
            for b in range(1, B):
                emit_p1(b)
                emit_p3h(b - 1, 0)
                emit_p2h(b, 0)
                emit_p3h(b - 1, 1)
                emit_p2h(b, 1)
            emit_p3h(B - 1, 0)
            emit_p3h(B - 1, 1)

    nc.compile()
    return nc


_NC_CACHE = None


def _get_nc():
    global _NC_CACHE
    if _NC_CACHE is None:
        _NC_CACHE = build_nc()
    return _NC_CACHE


def prep_inputs(x, w1, b1, w2, b2, w_post, b_post):
    """Host-side rearrangement of weights; returns per-core in_maps."""
    import ml_dtypes
    x = np.asarray(x, dtype=np.float32)
    n = x.shape[0]
    w1r = np.ascontiguousarray(
        np.asarray(w1, np.float32).transpose(1, 2, 0).reshape(C, TAPS * K))
    w1r = np.concatenate([w1r, np.zeros((C, 256 - TAPS * K), np.float32)], axis=1)
    w2r = np.ascontiguousarray(
        np.asarray(w2, np.float32).transpose(1, 2, 0).reshape(P, TAPS * K))
    w2r = np.concatenate([w2r, np.zeros((P, 256 - TAPS * K), np.float32)], axis=1)
    wpt = np.ascontiguousarray(
        np.asarray(w_post, np.float32).T).astype(ml_dtypes.bfloat16)
    b2c = np.asarray(b2, np.float32).reshape(K, 1)
    bpc = np.ascontiguousarray(
        np.asarray(b_post, np.float32).reshape(CT, 128).T)
    # expb1[i, pl*K + k] = exp(b1[k]) for every partition i, half-window pl
    eb1 = np.exp(np.asarray(b1, np.float32))
    expb1 = np.ascontiguousarray(
        np.broadcast_to(np.tile(eb1, HPW)[None, :], (128, HPW * K)))
    ident = np.eye(128, dtype=np.float32)
    identb = ident.astype(ml_dtypes.bfloat16)
    sel = np.zeros((32, 32), np.float32)
    for i in range(16):
        sel[i, i] = 1.0
        sel[16 + i, 16 + i] = 1.0
    selt = sel.astype(ml_dtypes.bfloat16)
    # xt[b, i, pw*C + c] = x[b, c, pw*128 + i]
    xt = np.ascontiguousarray(
        x.reshape(n, C, PW, 128).transpose(0, 3, 2, 1).reshape(n, 128, PW * C))
    consts = {"w1r": w1r, "w2r": w2r, "wpt": wpt, "b2c": b2c,
              "bpc": bpc, "expb1": expb1, "ident": ident, "identb": identb,
              "sel": sel, "selt": selt}
    in_maps = []
    for core in range(N_CORES):
        m = dict(consts)
        m["xs"] = np.ascontiguousarray(x[core * B:(core + 1) * B])
        m["xt"] = np.ascontiguousarray(xt[core * B:(core + 1) * B])
        in_maps.append(m)
    return in_maps


def run(inputs, trace=False):
    import os
    if not trace:
        # the axon NTFF profile hook is unavailable in this container; a
        # stray BASS_TRACE=1 in the environment would crash the run
        os.environ["BASS_NEVER_TRACE"] = "1"
    nc = _get_nc()
    in_maps = prep_inputs(**inputs)
    res = run_bass_kernel_spmd(nc, in_maps, list(range(N_CORES)), trace=trace)
    out = np.concatenate([res.results[i]["out"] for i in range(N_CORES)], axis=0)
    return out.astype(np.float32), res


def kernel(x, w1, b1, w2, b2, w_post, b_post):
    out, _ = run(dict(x=x, w1=w1, b1=b1, w2=w2, b2=b2,
                      w_post=w_post, b_post=b_post))
    return out


# revision 32
# speedup vs baseline: 1.0283x; 1.0283x over previous
"""ClusterAttention TRN2 kernel (v3).

Computation (per batch item):
  a  = conv1d_same(x, w1, b1)                      # (k, p) conv over p
  bm = conv1d_same(x^T, w2, b2)                    # (k, c) conv over c
  bm = bm @ x                                      # (k, p)
  probs = softmax(a + bm, axis=k)                  # (k, p)
  out = w_post @ (probs[:, None, :] * x[None]).reshape(k*c, p) + b_post

Sharding: data-parallel over batch n (32) across 8 cores, 4 per core.
All params replicated.

v3 design:
  - convs as dense matmuls with taps stacked on output partitions, then a
    log-tree of shifted adds on DVE (a-tree in bf16, bm-tree in f32)
  - logits computed DIRECTLY transposed: pa4T[p, k] per 128-wide p-window
    via selector matmuls (stationary = tree output D, moving = selT) plus
    x^T @ bmT matmuls; softmax runs on [p, k] tiles with no PE transposes
  - softmax normalization batched per half: one reciprocal + wide DVE
    multiplies (free-dim broadcast APs) instead of per-window ops
  - probs -> DRAM via XBAR dma transpose; broadcast back to [128, p] tiles
    with stride-0 partition-broadcast DMA reads (no gpsimd)
  - xT uploaded from host; bias adds on Act; consts loaded in few DMAs
"""
import sys

sys.path.insert(0, "/opt/trn_rl_repo")

import numpy as np

import concourse.bass as bass
import concourse.mybir as mybir
import concourse.tile as tile
from concourse import bacc
from concourse.bass_utils import run_bass_kernel_spmd

dt = mybir.dt

N_CORES = 8
B = 4            # batch items per core
C = 256          # channels
P = 2048         # sequence length
K = 16           # clusters
TAPS = 15
PAD = 7
CT = C // 128    # c-tiles
NCH = P // 512   # 512-wide p-chunks
PW = P // 128    # 128-wide p-windows
HPW = PW // 2    # p-windows per half
KG = 2           # clusters per broadcast group

AW = P + 16      # A_buf width: col j holds A[p = j - PAD], zeros outside
BMW = C + 16     # BM_buf width


def build_nc():
    nc = bacc.Bacc(None)

    xs_d = nc.dram_tensor("xs", [B, C, P], dt.float32, kind="ExternalInput")
    xt_d = nc.dram_tensor("xt", [B, 128, PW * C], dt.float32, kind="ExternalInput")
    w1r_d = nc.dram_tensor("w1r", [C, 256], dt.float32, kind="ExternalInput")
    w2r_d = nc.dram_tensor("w2r", [P, 256], dt.float32, kind="ExternalInput")
    wpt_d = nc.dram_tensor("wpt", [K * C, C], dt.bfloat16, kind="ExternalInput")
    b2_d = nc.dram_tensor("b2c", [K, 1], dt.float32, kind="ExternalInput")
    bp_d = nc.dram_tensor("bpc", [128, CT], dt.float32, kind="ExternalInput")
    eb1_d = nc.dram_tensor("expb1", [128, HPW * K], dt.float32, kind="ExternalInput")
    id_d = nc.dram_tensor("ident", [128, 128], dt.float32, kind="ExternalInput")
    sel_d = nc.dram_tensor("sel", [32, 32], dt.float32, kind="ExternalInput")
    selt_d = nc.dram_tensor("selt", [32, 32], dt.bfloat16, kind="ExternalInput")
    idb_d = nc.dram_tensor("identb", [128, 128], dt.bfloat16, kind="ExternalInput")
    out_d = nc.dram_tensor("out", [B, C, P], dt.float32, kind="ExternalOutput")

    F32R = dt.float32r
    BF16 = dt.bfloat16
    IDENT = mybir.ActivationFunctionType.Identity
    EXP = mybir.ActivationFunctionType.Exp

    with tile.TileContext(nc) as tc:
        with (
            tc.tile_pool(name="const", bufs=1) as cpool,
            tc.tile_pool(name="xs", bufs=2) as xpool,
            tc.tile_pool(name="xt", bufs=2) as xtpool,
            tc.tile_pool(name="abuf", bufs=1) as apool,
            tc.tile_pool(name="tree", bufs=2) as tpool,
            tc.tile_pool(name="sm", bufs=2) as smpool,
            tc.tile_pool(name="main", bufs=4) as mpool,
            tc.tile_pool(name="bc", bufs=4) as bcpool,
            tc.tile_pool(name="oc", bufs=2) as ocpool,
            tc.tile_pool(name="pd", bufs=2, space="DRAM") as pdpool,
            tc.tile_pool(name="pconv", bufs=2, space="PSUM") as pconv,
            tc.tile_pool(name="ptr", bufs=1, space="PSUM") as ptr,
            tc.tile_pool(name="pout", bufs=4, space="PSUM") as pout,
        ):
            # ---- constants (few, consolidated DMAs) ----
            w1rt = cpool.tile([128, 2, 256], F32R, tag="w1rt")
            nc.sync.dma_start(
                out=w1rt[:],
                in_=w1r_d.rearrange("(t i) c -> i t c", t=2).bitcast(F32R))
            ident = cpool.tile([128, 128], dt.float32, tag="ident")
            nc.sync.dma_start(out=ident[:], in_=id_d[:])
            sel = cpool.tile([32, 32], F32R, tag="sel")
            nc.sync.dma_start(out=sel[:], in_=sel_d[:].bitcast(F32R))
            selt = cpool.tile([32, 32], BF16, tag="selt")
            nc.sync.dma_start(out=selt[:], in_=selt_d[:])
            w2rt = cpool.tile([128, PW, 256], F32R, tag="w2rt")
            wptt = cpool.tile([128, 2 * K, 256], BF16, tag="wptt")
            b2c = cpool.tile([K, 1], dt.float32, tag="b2c")
            nc.sync.dma_start(out=b2c[:], in_=b2_d[:])
            bpc = cpool.tile([128, CT], dt.float32, tag="bpc")
            nc.sync.dma_start(out=bpc[:], in_=bp_d[:])
            expb1 = cpool.tile([128, HPW * K], dt.float32, tag="expb1")
            nc.sync.dma_start(out=expb1[:], in_=eb1_d[:])

            state = {}

            def emit_p1(b):
                """loads, xb convert, convs, trees, bm merge, bmT."""
                # ---- load x and xT ----
                xs = [xpool.tile([128, P], F32R, tag=f"xs{ct}", name=f"xs{ct}")
                      for ct in range(CT)]
                for ct in range(CT):
                    for hh in range(2):
                        nc.sync.dma_start(
                            out=xs[ct][:, hh * 1024:(hh + 1) * 1024],
                            in_=xs_d[b, ct * 128:(ct + 1) * 128,
                                     hh * 1024:(hh + 1) * 1024].bitcast(F32R))
                xT = xtpool.tile([128, PW * C], F32R, tag="xT", name="xT")
                for hh in range(4):
                    nc.sync.dma_start(
                        out=xT[:, hh * 1024:(hh + 1) * 1024],
                        in_=xt_d[b, :, hh * 1024:(hh + 1) * 1024].bitcast(F32R))
                if b == 0:
                    for hh in range(4):
                        nc.sync.dma_start(
                            out=w2rt[:, hh * 4:(hh + 1) * 4, :],
                            in_=w2r_d[hh * 512:(hh + 1) * 512]
                            .rearrange("(t i) c -> i t c", t=4).bitcast(F32R))

                # ---- bf16 copy of x for the y path ----
                xb = [xpool.tile([128, P], BF16, tag=f"xb{ct}",
                                 name=f"xb{ct}", bufs=3) for ct in range(CT)]
                for ct in range(CT):
                    nc.scalar.copy(xb[ct][:], xs[ct][:].bitcast(dt.float32))

                # ---- A conv matmuls: A[(t,kk), p] (bf16 store) ----
                A = [apool.tile([128, AW], BF16, tag=f"A{mc}", name=f"A{mc}")
                     for mc in range(2)]
                for mc in range(2):
                    nc.vector.memset(A[mc][:, 0:PAD], 0.0)
                    nc.vector.memset(A[mc][:, PAD + P:AW], 0.0)
                for mc in range(2):
                    for nch in range(NCH):
                        pa = pconv.tile([128, 512], dt.float32, tag="pconv")
                        for ct in range(CT):
                            nc.tensor.matmul(
                                pa[:],
                                w1rt[:, ct, mc * 128:(mc + 1) * 128],
                                xs[ct][:, nch * 512:(nch + 1) * 512],
                                start=(ct == 0), stop=(ct == CT - 1))
                        nc.scalar.copy(
                            A[mc][:, PAD + nch * 512:PAD + (nch + 1) * 512], pa[:])

                # ---- BM conv matmuls: BM[(t,kk), cc] (f32) ----
                BM = [apool.tile([128, BMW], dt.float32, tag=f"BM{mc}",
                                 name=f"BM{mc}") for mc in range(2)]
                for mc in range(2):
                    nc.vector.memset(BM[mc][:, 0:PAD], 0.0)
                    nc.vector.memset(BM[mc][:, PAD + C:BMW], 0.0)
                for mc in range(2):
                    pb = pconv.tile([128, C], dt.float32, tag="pconv")
                    for pw in range(PW):
                        nc.tensor.matmul(
                            pb[:],
                            w2rt[:, pw, mc * 128:(mc + 1) * 128],
                            xT[:, pw * C:pw * C + C],
                            start=(pw == 0), stop=(pw == PW - 1))
                    nc.scalar.copy(BM[mc][:, PAD:PAD + C], pb[:])

                # ---- shift-add tree for a (bf16) ----
                HALF = P // 2
                Da = []
                for h in range(2):
                    o = h * HALF
                    w1_, w2_, w3_ = HALF + 8, HALF + 4, HALF + 2
                    Bl = tpool.tile([64, w1_], BF16, tag="tr", name="Bl")
                    Bh = tpool.tile([64, w1_], BF16, tag="tr", name="Bh")
                    nc.vector.tensor_add(Bl[:], A[0][0:64, o:o + w1_],
                                         A[1][0:64, o + 8:o + 8 + w1_])
                    nc.vector.tensor_add(Bh[:], A[0][64:128, o:o + w1_],
                                         A[1][64:128, o + 8:o + 8 + w1_])
                    Cl = tpool.tile([32, w2_], BF16, tag="trc", name="Cl")
                    Ch = tpool.tile([32, w2_], BF16, tag="trc", name="Ch")
                    nc.vector.tensor_add(Cl[:], Bl[0:32, 0:w2_],
                                         Bh[0:32, 4:4 + w2_])
                    nc.vector.tensor_add(Ch[:], Bl[32:64, 0:w2_],
                                         Bh[32:64, 4:4 + w2_])
                    D = tpool.tile([32, w3_], BF16, tag="trd", name="D", bufs=2)
                    nc.vector.tensor_add(D[:], Cl[0:32, 0:w3_],
                                         Ch[0:32, 2:2 + w3_])
                    Da.append(D)

                # ---- shift-add tree for bm: [16, C] (f32) ----
                bmt = smpool.tile([K, C], dt.float32, tag="bmt")
                w1_, w2_, w3_ = C + 8, C + 4, C + 2
                Bl = tpool.tile([64, w1_], dt.float32, tag="trb", name="Blb")
                Bh = tpool.tile([64, w1_], dt.float32, tag="trb", name="Bhb")
                nc.vector.tensor_add(Bl[:], BM[0][0:64, 0:w1_],
                                     BM[1][0:64, 8:8 + w1_])
                nc.vector.tensor_add(Bh[:], BM[0][64:128, 0:w1_],
                                     BM[1][64:128, 8:8 + w1_])
                Cl = tpool.tile([32, w2_], dt.float32, tag="trcb", name="Clb")
                Ch = tpool.tile([32, w2_], dt.float32, tag="trcb", name="Chb")
                nc.vector.tensor_add(Cl[:], Bl[0:32, 0:w2_], Bh[0:32, 4:4 + w2_])
                nc.vector.tensor_add(Ch[:], Bl[32:64, 0:w2_], Bh[32:64, 4:4 + w2_])
                Db = tpool.tile([32, w3_], F32R, tag="trdb", name="Db", bufs=2)
                nc.vector.tensor_add(Db[:], Cl[0:32, 0:w3_],
                                     Ch[0:32, 2:2 + w3_])
                pb4 = pconv.tile([K, C], dt.float32, tag="pconv", name="pb4")
                nc.tensor.matmul(pb4[:], sel[:, 0:16], Db[:, 0:C],
                                 start=True, stop=False)
                nc.tensor.matmul(pb4[:], sel[:, 16:32], Db[:, 1:1 + C],
                                 start=False, stop=True)
                nc.scalar.activation(bmt[:], pb4[:], IDENT, bias=b2c[:])

                # ---- bmT = bm^T [C, 16] (2 tiles) ----
                bmT = [smpool.tile([128, K], F32R, tag=f"bmT{ct}", name=f"bmT{ct}")
                       for ct in range(CT)]
                for ct in range(CT):
                    pt = pconv.tile([128, 512], dt.float32, tag="pconv", name="pt")
                    nc.tensor.transpose(
                        pt[0:128, 0:K], bmt[:, ct * 128:(ct + 1) * 128],
                        ident[0:K, 0:K])
                    nc.scalar.copy(bmT[ct][:], pt[0:128, 0:K])
                state[b] = dict(xs=xs, xb=xb, Da=Da, bmT=bmT)

            def emit_p2h(b, h):
                """transposed logits + softmax + probs -> DRAM for one half."""
                xs = state[b]["xs"]
                D = state[b]["Da"][h]
                bmT = state[b]["bmT"]
                if h == 0:
                    state[b]["probs_d"] = pdpool.tile(
                        [K, P], BF16, tag="probs_d", name="probs_d")
                probs_d = state[b]["probs_d"]
                # pa4T[p, k] = a-merge (selector mms on D) + x^T @ bmT
                expTh = smpool.tile([128, HPW * K], dt.float32,
                                    tag="expTh", name="expTh")
                esums = smpool.tile([128, HPW], dt.float32, tag="esums",
                                    name="esums")
                for pl in range(HPW):
                    pw = h * HPW + pl
                    o = pl * 128
                    pa4T = ptr.tile([128, K], dt.float32, tag="pa4T",
                                    bufs=2)
                    nc.tensor.matmul(pa4T[:], D[:, o:o + 128],
                                     selt[:, 0:16], start=True, stop=False)
                    nc.tensor.matmul(pa4T[:], D[:, o + 1:o + 129],
                                     selt[:, 16:32], start=False, stop=False)
                    for ct in range(CT):
                        nc.tensor.matmul(
                            pa4T[:],
                            xs[ct][:, pw * 128:(pw + 1) * 128],
                            bmT[ct][:],
                            start=False, stop=(ct == CT - 1))
                    negmx = smpool.tile([128, 1], dt.float32, tag="negmx")
                    nc.vector.tensor_reduce(
                        negmx[:], pa4T[:], axis=mybir.AxisListType.X,
                        op=mybir.AluOpType.max, negate=True)
                    nc.scalar.activation(
                        expTh[:, pl * K:(pl + 1) * K], pa4T[:],
                        EXP, bias=negmx[:])
                # esums/normalize batched over the half
                tmpE = smpool.tile([128, HPW * K], dt.float32, tag="tmpE",
                                   name="tmpE")
                nc.vector.tensor_mul(tmpE[:], expTh[:], expb1[:])
                nc.vector.tensor_reduce(
                    esums[:], tmpE[:].rearrange("p (a b) -> p a b", a=HPW),
                    axis=mybir.AxisListType.X, op=mybir.AluOpType.add)
                recips = smpool.tile([128, HPW], dt.float32, tag="recips",
                                     name="recips")
                nc.vector.reciprocal(recips[:], esums[:])
                pTh = smpool.tile([128, HPW * K], dt.float32, tag="probsTh",
                                  name="probsTh")
                nc.vector.tensor_mul(
                    pTh[:].rearrange("p (a b) -> p a b", a=HPW),
                    tmpE[:].rearrange("p (a b) -> p a b", a=HPW),
                    recips[:].unsqueeze(2).to_broadcast([128, HPW, K]))
                # probs -> DRAM: ppb[(a k), i] = pTh[i, a*K + k]; the DMA
                # scatters partition a*K+k to probs_d[k, h*1024 + a*128 + i]
                ppb = pconv.tile([128, 512], dt.float32, tag="pconv",
                                 name="ppb")
                nc.tensor.transpose(ppb[0:128, 0:128], pTh[:], ident[:])
                probsS = smpool.tile([128, 128], BF16, tag="probsS",
                                     name="probsS")
                nc.scalar.copy(probsS[:], ppb[0:128, 0:128])
                nc.scalar.dma_start(
                    out=probs_d[:, h * 1024:(h + 1) * 1024]
                    .rearrange("k (a i) -> a k i", a=HPW),
                    in_=probsS[:])
                bcts = []
                for kg in range(K // KG):
                    bct = bcpool.tile([128, KG, 1024], BF16,
                                      tag="bct", name="bct")
                    nc.sync.dma_start(
                        out=bct[:],
                        in_=probs_d[kg * KG:(kg + 1) * KG,
                                    h * 1024:(h + 1) * 1024]
                        .unsqueeze(0).partition_broadcast(128))
                    bcts.append(bct)
                state[b][f"bcts{h}"] = bcts
                if b == 0 and h == 0:
                    for hh in range(4):
                        nc.sync.dma_start(
                            out=wptt[:, hh * 8:(hh + 1) * 8, :],
                            in_=wpt_d[hh * 1024:(hh + 1) * 1024]
                            .rearrange("(t i) c -> i t c", t=8))

            def emit_p3h(b, half):
                """y build, main matmul, bias + store for one half."""
                xb = state[b]["xb"]
                chunks = [2 * half, 2 * half + 1]
                po = {}
                for nch in chunks:
                    for ot in range(CT):
                        po[(nch, ot)] = pout.tile(
                            [128, 512], dt.float32, tag="pout", name="po")
                for kg in range(K // KG):
                    bct = state[b][f"bcts{half}"][kg]
                    for j in range(KG):
                        kk = kg * KG + j
                        ys = []
                        for ct in range(CT):
                            y = mpool.tile([128, 1024], BF16,
                                           tag="y", name="y")
                            nc.vector.tensor_mul(
                                y[:],
                                xb[ct][:, half * 1024:(half + 1) * 1024],
                                bct[:, j, :])
                            ys.append(y)
                        for nch in chunks:
                            co = (nch % 2) * 512
                            for ct in range(CT):
                                for ot in range(CT):
                                    nc.tensor.matmul(
                                        po[(nch, ot)][:],
                                        wptt[:, kk * 2 + ct,
                                             ot * 128:(ot + 1) * 128],
                                        ys[ct][:, co:co + 512],
                                        start=(kg == 0 and j == 0 and ct == 0),
                                        stop=(kg == K // KG - 1 and j == KG - 1
                                              and ct == CT - 1))
                for ot in range(CT):
                    oc = ocpool.tile([128, 1024], dt.float32, tag="oc",
                                     name="oc")
                    for nch in chunks:
                        nc.scalar.activation(
                            oc[:, (nch % 2) * 512:(nch % 2) * 512 + 512],
                            po[(nch, ot)][:], IDENT,
                            bias=bpc[:, ot:ot + 1])
                    nc.scalar.dma_start(
                        out=out_d[b, ot * 128:(ot + 1) * 128,
                                  half * 1024:(half + 1) * 1024],
                        in_=oc[:])

            emit_p1(0)
            emit_p2h(0, 0)
            emit_p2h(0, 1)
            for b in range(1, B):
                emit_p1(b)
                emit_p3h(b - 1, 0)
                emit_p2h(b, 0)
                emit_p3h(b - 1, 1)
                emit_p2h(b, 1)
            emit_p3h(B - 1, 0)
            emit_p3h(B - 1, 1)

    nc.compile()
    return nc


_NC_CACHE = None


def _get_nc():
    global _NC_CACHE
    if _NC_CACHE is None:
        _NC_CACHE = build_nc()
    return _NC_CACHE


def prep_inputs(x, w1, b1, w2, b2, w_post, b_post):
    """Host-side rearrangement of weights; returns per-core in_maps."""
    import ml_dtypes
    x = np.asarray(x, dtype=np.float32)
    n = x.shape[0]
    w1r = np.ascontiguousarray(
        np.asarray(w1, np.float32).transpose(1, 2, 0).reshape(C, TAPS * K))
    w1r = np.concatenate([w1r, np.zeros((C, 256 - TAPS * K), np.float32)], axis=1)
    w2r = np.ascontiguousarray(
        np.asarray(w2, np.float32).transpose(1, 2, 0).reshape(P, TAPS * K))
    w2r = np.concatenate([w2r, np.zeros((P, 256 - TAPS * K), np.float32)], axis=1)
    wpt = np.ascontiguousarray(
        np.asarray(w_post, np.float32).T).astype(ml_dtypes.bfloat16)
    b2c = np.asarray(b2, np.float32).reshape(K, 1)
    bpc = np.ascontiguousarray(
        np.asarray(b_post, np.float32).reshape(CT, 128).T)
    # expb1[i, pl*K + k] = exp(b1[k]) for every partition i, half-window pl
    eb1 = np.exp(np.asarray(b1, np.float32))
    expb1 = np.ascontiguousarray(
        np.broadcast_to(np.tile(eb1, HPW)[None, :], (128, HPW * K)))
    ident = np.eye(128, dtype=np.float32)
    identb = ident.astype(ml_dtypes.bfloat16)
    sel = np.zeros((32, 32), np.float32)
    for i in range(16):
        sel[i, i] = 1.0
        sel[16 + i, 16 + i] = 1.0
    selt = sel.astype(ml_dtypes.bfloat16)
    # xt[b, i, pw*C + c] = x[b, c, pw*128 + i]
    xt = np.ascontiguousarray(
        x.reshape(n, C, PW, 128).transpose(0, 3, 2, 1).reshape(n, 128, PW * C))
    consts = {"w1r": w1r, "w2r": w2r, "wpt": wpt, "b2c": b2c,
              "bpc": bpc, "expb1": expb1, "ident": ident, "identb": identb,
              "sel": sel, "selt": selt}
    in_maps = []
    for core in range(N_CORES):
        m = dict(consts)
        m["xs"] = np.ascontiguousarray(x[core * B:(core + 1) * B])
        m["xt"] = np.ascontiguousarray(xt[core * B:(core + 1) * B])
        in_maps.append(m)
    return in_maps


def run(inputs, trace=False):
    import os
    if not trace:
        # the axon NTFF profile hook is unavailable in this container; a
        # stray BASS_TRACE=1 in the environment would crash the run
        os.environ["BASS_NEVER_TRACE"] = "1"
    nc = _get_nc()
    in_maps = prep_inputs(**inputs)
    res = run_bass_kernel_spmd(nc, in_maps, list(range(N_CORES)), trace=trace)
    out = np.concatenate([res.results[i]["out"] for i in range(N_CORES)], axis=0)
    return out.astype(np.float32), res


def kernel(x, w1, b1, w2, b2, w_post, b_post):
    out, _ = run(dict(x=x, w1=w1, b1=b1, w2=w2, b2=b2,
                      w_post=w_post, b_post=b_post))
    return out


# revision 33
# speedup vs baseline: 1.0465x; 1.0177x over previous
"""ClusterAttention TRN2 kernel (v3).

Computation (per batch item):
  a  = conv1d_same(x, w1, b1)                      # (k, p) conv over p
  bm = conv1d_same(x^T, w2, b2)                    # (k, c) conv over c
  bm = bm @ x                                      # (k, p)
  probs = softmax(a + bm, axis=k)                  # (k, p)
  out = w_post @ (probs[:, None, :] * x[None]).reshape(k*c, p) + b_post

Sharding: data-parallel over batch n (32) across 8 cores, 4 per core.
All params replicated.

v3 design:
  - convs as dense matmuls with taps stacked on output partitions, then a
    log-tree of shifted adds on DVE (a-tree in bf16, bm-tree in f32)
  - logits computed DIRECTLY transposed: pa4T[p, k] per 128-wide p-window
    via selector matmuls (stationary = tree output D, moving = selT) plus
    x^T @ bmT matmuls; softmax runs on [p, k] tiles with no PE transposes
  - softmax normalization batched per half: one reciprocal + wide DVE
    multiplies (free-dim broadcast APs) instead of per-window ops
  - probs -> DRAM via XBAR dma transpose; broadcast back to [128, p] tiles
    with stride-0 partition-broadcast DMA reads (no gpsimd)
  - xT uploaded from host; bias adds on Act; consts loaded in few DMAs
"""
import sys

sys.path.insert(0, "/opt/trn_rl_repo")

import numpy as np

import concourse.bass as bass
import concourse.mybir as mybir
import concourse.tile as tile
from concourse import bacc
from concourse.bass_utils import run_bass_kernel_spmd

dt = mybir.dt

N_CORES = 8
B = 4            # batch items per core
C = 256          # channels
P = 2048         # sequence length
K = 16           # clusters
TAPS = 15
PAD = 7
CT = C // 128    # c-tiles
NCH = P // 512   # 512-wide p-chunks
PW = P // 128    # 128-wide p-windows
HPW = PW // 2    # p-windows per half
KG = 2           # clusters per broadcast group

AW = P + 16      # A_buf width: col j holds A[p = j - PAD], zeros outside
BMW = C + 16     # BM_buf width


def build_nc():
    nc = bacc.Bacc(None)

    xs_d = nc.dram_tensor("xs", [B, C, P], dt.float32, kind="ExternalInput")
    xt_d = nc.dram_tensor("xt", [B, 128, PW * C], dt.float32, kind="ExternalInput")
    w1r_d = nc.dram_tensor("w1r", [C, 256], dt.float32, kind="ExternalInput")
    w2r_d = nc.dram_tensor("w2r", [P, 256], dt.float32, kind="ExternalInput")
    wpt_d = nc.dram_tensor("wpt", [K * C, C], dt.bfloat16, kind="ExternalInput")
    b2_d = nc.dram_tensor("b2c", [K, 1], dt.float32, kind="ExternalInput")
    bp_d = nc.dram_tensor("bpc", [128, CT], dt.float32, kind="ExternalInput")
    eb1_d = nc.dram_tensor("expb1", [128, HPW * K], dt.float32, kind="ExternalInput")
    id_d = nc.dram_tensor("ident", [128, 128], dt.float32, kind="ExternalInput")
    sel_d = nc.dram_tensor("sel", [32, 32], dt.float32, kind="ExternalInput")
    selt_d = nc.dram_tensor("selt", [32, 32], dt.bfloat16, kind="ExternalInput")
    idb_d = nc.dram_tensor("identb", [128, 128], dt.bfloat16, kind="ExternalInput")
    out_d = nc.dram_tensor("out", [B, C, P], dt.float32, kind="ExternalOutput")

    F32R = dt.float32r
    BF16 = dt.bfloat16
    IDENT = mybir.ActivationFunctionType.Identity
    EXP = mybir.ActivationFunctionType.Exp

    with tile.TileContext(nc) as tc:
        with (
            tc.tile_pool(name="const", bufs=1) as cpool,
            tc.tile_pool(name="xs", bufs=2) as xpool,
            tc.tile_pool(name="xt", bufs=2) as xtpool,
            tc.tile_pool(name="abuf", bufs=1) as apool,
            tc.tile_pool(name="tree", bufs=2) as tpool,
            tc.tile_pool(name="sm", bufs=2) as smpool,
            tc.tile_pool(name="main", bufs=6) as mpool,
            tc.tile_pool(name="bc", bufs=6) as bcpool,
            tc.tile_pool(name="oc", bufs=2) as ocpool,
            tc.tile_pool(name="pd", bufs=2, space="DRAM") as pdpool,
            tc.tile_pool(name="pconv", bufs=2, space="PSUM") as pconv,
            tc.tile_pool(name="ptr", bufs=1, space="PSUM") as ptr,
            tc.tile_pool(name="pout", bufs=4, space="PSUM") as pout,
        ):
            # ---- constants (few, consolidated DMAs) ----
            w1rt = cpool.tile([128, 2, 256], F32R, tag="w1rt")
            nc.sync.dma_start(
                out=w1rt[:],
                in_=w1r_d.rearrange("(t i) c -> i t c", t=2).bitcast(F32R))
            ident = cpool.tile([128, 128], dt.float32, tag="ident")
            nc.sync.dma_start(out=ident[:], in_=id_d[:])
            sel = cpool.tile([32, 32], F32R, tag="sel")
            nc.sync.dma_start(out=sel[:], in_=sel_d[:].bitcast(F32R))
            selt = cpool.tile([32, 32], BF16, tag="selt")
            nc.sync.dma_start(out=selt[:], in_=selt_d[:])
            w2rt = cpool.tile([128, PW, 256], F32R, tag="w2rt")
            wptt = cpool.tile([128, 2 * K, 256], BF16, tag="wptt")
            b2c = cpool.tile([K, 1], dt.float32, tag="b2c")
            nc.sync.dma_start(out=b2c[:], in_=b2_d[:])
            bpc = cpool.tile([128, CT], dt.float32, tag="bpc")
            nc.sync.dma_start(out=bpc[:], in_=bp_d[:])
            expb1 = cpool.tile([128, HPW * K], dt.float32, tag="expb1")
            nc.sync.dma_start(out=expb1[:], in_=eb1_d[:])

            state = {}

            def emit_p1(b):
                """loads, xb convert, convs, trees, bm merge, bmT."""
                # ---- load x and xT ----
                xs = [xpool.tile([128, P], F32R, tag=f"xs{ct}", name=f"xs{ct}")
                      for ct in range(CT)]
                for ct in range(CT):
                    for hh in range(2):
                        nc.sync.dma_start(
                            out=xs[ct][:, hh * 1024:(hh + 1) * 1024],
                            in_=xs_d[b, ct * 128:(ct + 1) * 128,
                                     hh * 1024:(hh + 1) * 1024].bitcast(F32R))
                xT = xtpool.tile([128, PW * C], F32R, tag="xT", name="xT")
                for hh in range(4):
                    nc.sync.dma_start(
                        out=xT[:, hh * 1024:(hh + 1) * 1024],
                        in_=xt_d[b, :, hh * 1024:(hh + 1) * 1024].bitcast(F32R))
                if b == 0:
                    for hh in range(4):
                        nc.sync.dma_start(
                            out=w2rt[:, hh * 4:(hh + 1) * 4, :],
                            in_=w2r_d[hh * 512:(hh + 1) * 512]
                            .rearrange("(t i) c -> i t c", t=4).bitcast(F32R))

                # ---- bf16 copy of x for the y path ----
                xb = [xpool.tile([128, P], BF16, tag=f"xb{ct}",
                                 name=f"xb{ct}", bufs=3) for ct in range(CT)]
                for ct in range(CT):
                    nc.scalar.copy(xb[ct][:], xs[ct][:].bitcast(dt.float32))

                # ---- A conv matmuls: A[(t,kk), p] (bf16 store) ----
                A = [apool.tile([128, AW], BF16, tag=f"A{mc}", name=f"A{mc}")
                     for mc in range(2)]
                for mc in range(2):
                    nc.vector.memset(A[mc][:, 0:PAD], 0.0)
                    nc.vector.memset(A[mc][:, PAD + P:AW], 0.0)
                for mc in range(2):
                    for nch in range(NCH):
                        pa = pconv.tile([128, 512], dt.float32, tag="pconv")
                        for ct in range(CT):
                            nc.tensor.matmul(
                                pa[:],
                                w1rt[:, ct, mc * 128:(mc + 1) * 128],
                                xs[ct][:, nch * 512:(nch + 1) * 512],
                                start=(ct == 0), stop=(ct == CT - 1))
                        nc.scalar.copy(
                            A[mc][:, PAD + nch * 512:PAD + (nch + 1) * 512], pa[:])

                # ---- BM conv matmuls: BM[(t,kk), cc] (f32) ----
                BM = [apool.tile([128, BMW], dt.float32, tag=f"BM{mc}",
                                 name=f"BM{mc}") for mc in range(2)]
                for mc in range(2):
                    nc.vector.memset(BM[mc][:, 0:PAD], 0.0)
                    nc.vector.memset(BM[mc][:, PAD + C:BMW], 0.0)
                for mc in range(2):
                    pb = pconv.tile([128, C], dt.float32, tag="pconv")
                    for pw in range(PW):
                        nc.tensor.matmul(
                            pb[:],
                            w2rt[:, pw, mc * 128:(mc + 1) * 128],
                            xT[:, pw * C:pw * C + C],
                            start=(pw == 0), stop=(pw == PW - 1))
                    nc.scalar.copy(BM[mc][:, PAD:PAD + C], pb[:])

                # ---- shift-add tree for a (bf16) ----
                HALF = P // 2
                Da = []
                for h in range(2):
                    o = h * HALF
                    w1_, w2_, w3_ = HALF + 8, HALF + 4, HALF + 2
                    Bl = tpool.tile([64, w1_], BF16, tag="tr", name="Bl")
                    Bh = tpool.tile([64, w1_], BF16, tag="tr", name="Bh")
                    nc.vector.tensor_add(Bl[:], A[0][0:64, o:o + w1_],
                                         A[1][0:64, o + 8:o + 8 + w1_])
                    nc.vector.tensor_add(Bh[:], A[0][64:128, o:o + w1_],
                                         A[1][64:128, o + 8:o + 8 + w1_])
                    Cl = tpool.tile([32, w2_], BF16, tag="trc", name="Cl")
                    Ch = tpool.tile([32, w2_], BF16, tag="trc", name="Ch")
                    nc.vector.tensor_add(Cl[:], Bl[0:32, 0:w2_],
                                         Bh[0:32, 4:4 + w2_])
                    nc.vector.tensor_add(Ch[:], Bl[32:64, 0:w2_],
                                         Bh[32:64, 4:4 + w2_])
                    D = tpool.tile([32, w3_], BF16, tag="trd", name="D", bufs=2)
                    nc.vector.tensor_add(D[:], Cl[0:32, 0:w3_],
                                         Ch[0:32, 2:2 + w3_])
                    Da.append(D)

                # ---- shift-add tree for bm: [16, C] (f32) ----
                bmt = smpool.tile([K, C], dt.float32, tag="bmt")
                w1_, w2_, w3_ = C + 8, C + 4, C + 2
                Bl = tpool.tile([64, w1_], dt.float32, tag="trb", name="Blb")
                Bh = tpool.tile([64, w1_], dt.float32, tag="trb", name="Bhb")
                nc.vector.tensor_add(Bl[:], BM[0][0:64, 0:w1_],
                                     BM[1][0:64, 8:8 + w1_])
                nc.vector.tensor_add(Bh[:], BM[0][64:128, 0:w1_],
                                     BM[1][64:128, 8:8 + w1_])
                Cl = tpool.tile([32, w2_], dt.float32, tag="trcb", name="Clb")
                Ch = tpool.tile([32, w2_], dt.float32, tag="trcb", name="Chb")
                nc.vector.tensor_add(Cl[:], Bl[0:32, 0:w2_], Bh[0:32, 4:4 + w2_])
                nc.vector.tensor_add(Ch[:], Bl[32:64, 0:w2_], Bh[32:64, 4:4 + w2_])
                Db = tpool.tile([32, w3_], F32R, tag="trdb", name="Db", bufs=2)
                nc.vector.tensor_add(Db[:], Cl[0:32, 0:w3_],
                                     Ch[0:32, 2:2 + w3_])
                pb4 = pconv.tile([K, C], dt.float32, tag="pconv", name="pb4")
                nc.tensor.matmul(pb4[:], sel[:, 0:16], Db[:, 0:C],
                                 start=True, stop=False)
                nc.tensor.matmul(pb4[:], sel[:, 16:32], Db[:, 1:1 + C],
                                 start=False, stop=True)
                nc.scalar.activation(bmt[:], pb4[:], IDENT, bias=b2c[:])

                # ---- bmT = bm^T [C, 16] (2 tiles) ----
                bmT = [smpool.tile([128, K], F32R, tag=f"bmT{ct}", name=f"bmT{ct}")
                       for ct in range(CT)]
                for ct in range(CT):
                    pt = pconv.tile([128, 512], dt.float32, tag="pconv", name="pt")
                    nc.tensor.transpose(
                        pt[0:128, 0:K], bmt[:, ct * 128:(ct + 1) * 128],
                        ident[0:K, 0:K])
                    nc.scalar.copy(bmT[ct][:], pt[0:128, 0:K])
                state[b] = dict(xs=xs, xb=xb, Da=Da, bmT=bmT)

            def emit_p2h(b, h):
                """transposed logits + softmax + probs -> DRAM for one half."""
                xs = state[b]["xs"]
                D = state[b]["Da"][h]
                bmT = state[b]["bmT"]
                if h == 0:
                    state[b]["probs_d"] = pdpool.tile(
                        [K, P], BF16, tag="probs_d", name="probs_d")
                probs_d = state[b]["probs_d"]
                # pa4T[p, k] = a-merge (selector mms on D) + x^T @ bmT
                expTh = smpool.tile([128, HPW * K], dt.float32,
                                    tag="expTh", name="expTh")
                esums = smpool.tile([128, HPW], dt.float32, tag="esums",
                                    name="esums")
                for pl in range(HPW):
                    pw = h * HPW + pl
                    o = pl * 128
                    pa4T = ptr.tile([128, K], dt.float32, tag="pa4T",
                                    bufs=2)
                    nc.tensor.matmul(pa4T[:], D[:, o:o + 128],
                                     selt[:, 0:16], start=True, stop=False)
                    nc.tensor.matmul(pa4T[:], D[:, o + 1:o + 129],
                                     selt[:, 16:32], start=False, stop=False)
                    for ct in range(CT):
                        nc.tensor.matmul(
                            pa4T[:],
                            xs[ct][:, pw * 128:(pw + 1) * 128],
                            bmT[ct][:],
                            start=False, stop=(ct == CT - 1))
                    negmx = smpool.tile([128, 1], dt.float32, tag="negmx")
                    nc.vector.tensor_reduce(
                        negmx[:], pa4T[:], axis=mybir.AxisListType.X,
                        op=mybir.AluOpType.max, negate=True)
                    nc.scalar.activation(
                        expTh[:, pl * K:(pl + 1) * K], pa4T[:],
                        EXP, bias=negmx[:])
                # esums/normalize batched over the half
                tmpE = smpool.tile([128, HPW * K], dt.float32, tag="tmpE",
                                   name="tmpE")
                nc.vector.tensor_mul(tmpE[:], expTh[:], expb1[:])
                nc.vector.tensor_reduce(
                    esums[:], tmpE[:].rearrange("p (a b) -> p a b", a=HPW),
                    axis=mybir.AxisListType.X, op=mybir.AluOpType.add)
                recips = smpool.tile([128, HPW], dt.float32, tag="recips",
                                     name="recips")
                nc.vector.reciprocal(recips[:], esums[:])
                pTh = smpool.tile([128, HPW * K], dt.float32, tag="probsTh",
                                  name="probsTh")
                nc.vector.tensor_mul(
                    pTh[:].rearrange("p (a b) -> p a b", a=HPW),
                    tmpE[:].rearrange("p (a b) -> p a b", a=HPW),
                    recips[:].unsqueeze(2).to_broadcast([128, HPW, K]))
                # probs -> DRAM: ppb[(a k), i] = pTh[i, a*K + k]; the DMA
                # scatters partition a*K+k to probs_d[k, h*1024 + a*128 + i]
                ppb = pconv.tile([128, 512], dt.float32, tag="pconv",
                                 name="ppb")
                nc.tensor.transpose(ppb[0:128, 0:128], pTh[:], ident[:])
                probsS = smpool.tile([128, 128], BF16, tag="probsS",
                                     name="probsS")
                nc.scalar.copy(probsS[:], ppb[0:128, 0:128])
                nc.scalar.dma_start(
                    out=probs_d[:, h * 1024:(h + 1) * 1024]
                    .rearrange("k (a i) -> a k i", a=HPW),
                    in_=probsS[:])
                bcts = []
                for kg in range(K // KG):
                    bct = bcpool.tile([128, KG, 1024], BF16,
                                      tag="bct", name="bct")
                    nc.sync.dma_start(
                        out=bct[:],
                        in_=probs_d[kg * KG:(kg + 1) * KG,
                                    h * 1024:(h + 1) * 1024]
                        .unsqueeze(0).partition_broadcast(128))
                    bcts.append(bct)
                state[b][f"bcts{h}"] = bcts
                if b == 0 and h == 0:
                    for hh in range(4):
                        nc.sync.dma_start(
                            out=wptt[:, hh * 8:(hh + 1) * 8, :],
                            in_=wpt_d[hh * 1024:(hh + 1) * 1024]
                            .rearrange("(t i) c -> i t c", t=8))

            def emit_p3h(b, half):
                """y build, main matmul, bias + store for one half."""
                xb = state[b]["xb"]
                chunks = [2 * half, 2 * half + 1]
                po = {}
                for nch in chunks:
                    for ot in range(CT):
                        po[(nch, ot)] = pout.tile(
                            [128, 512], dt.float32, tag="pout", name="po")
                for kg in range(K // KG):
                    bct = state[b][f"bcts{half}"][kg]
                    for j in range(KG):
                        kk = kg * KG + j
                        ys = []
                        for ct in range(CT):
                            y = mpool.tile([128, 1024], BF16,
                                           tag="y", name="y")
                            nc.vector.tensor_mul(
                                y[:],
                                xb[ct][:, half * 1024:(half + 1) * 1024],
                                bct[:, j, :])
                            ys.append(y)
                        for nch in chunks:
                            co = (nch % 2) * 512
                            for ct in range(CT):
                                for ot in range(CT):
                                    nc.tensor.matmul(
                                        po[(nch, ot)][:],
                                        wptt[:, kk * 2 + ct,
                                             ot * 128:(ot + 1) * 128],
                                        ys[ct][:, co:co + 512],
                                        start=(kg == 0 and j == 0 and ct == 0),
                                        stop=(kg == K // KG - 1 and j == KG - 1
                                              and ct == CT - 1))
                for ot in range(CT):
                    oc = ocpool.tile([128, 1024], dt.float32, tag="oc",
                                     name="oc")
                    for nch in chunks:
                        nc.scalar.activation(
                            oc[:, (nch % 2) * 512:(nch % 2) * 512 + 512],
                            po[(nch, ot)][:], IDENT,
                            bias=bpc[:, ot:ot + 1])
                    nc.scalar.dma_start(
                        out=out_d[b, ot * 128:(ot + 1) * 128,
                                  half * 1024:(half + 1) * 1024],
                        in_=oc[:])

            emit_p1(0)
            emit_p2h(0, 0)
            emit_p2h(0, 1)
            for b in range(1, B):
                emit_p1(b)
                emit_p3h(b - 1, 0)
                emit_p2h(b, 0)
                emit_p3h(b - 1, 1)
                emit_p2h(b, 1)
            emit_p3h(B - 1, 0)
            emit_p3h(B - 1, 1)

    nc.compile()
    return nc


_NC_CACHE = None


def _get_nc():
    global _NC_CACHE
    if _NC_CACHE is None:
        _NC_CACHE = build_nc()
    return _NC_CACHE


def prep_inputs(x, w1, b1, w2, b2, w_post, b_post):
    """Host-side rearrangement of weights; returns per-core in_maps."""
    import ml_dtypes
    x = np.asarray(x, dtype=np.float32)
    n = x.shape[0]
    w1r = np.ascontiguousarray(
        np.asarray(w1, np.float32).transpose(1, 2, 0).reshape(C, TAPS * K))
    w1r = np.concatenate([w1r, np.zeros((C, 256 - TAPS * K), np.float32)], axis=1)
    w2r = np.ascontiguousarray(
        np.asarray(w2, np.float32).transpose(1, 2, 0).reshape(P, TAPS * K))
    w2r = np.concatenate([w2r, np.zeros((P, 256 - TAPS * K), np.float32)], axis=1)
    wpt = np.ascontiguousarray(
        np.asarray(w_post, np.float32).T).astype(ml_dtypes.bfloat16)
    b2c = np.asarray(b2, np.float32).reshape(K, 1)
    bpc = np.ascontiguousarray(
        np.asarray(b_post, np.float32).reshape(CT, 128).T)
    # expb1[i, pl*K + k] = exp(b1[k]) for every partition i, half-window pl
    eb1 = np.exp(np.asarray(b1, np.float32))
    expb1 = np.ascontiguousarray(
        np.broadcast_to(np.tile(eb1, HPW)[None, :], (128, HPW * K)))
    ident = np.eye(128, dtype=np.float32)
    identb = ident.astype(ml_dtypes.bfloat16)
    sel = np.zeros((32, 32), np.float32)
    for i in range(16):
        sel[i, i] = 1.0
        sel[16 + i, 16 + i] = 1.0
    selt = sel.astype(ml_dtypes.bfloat16)
    # xt[b, i, pw*C + c] = x[b, c, pw*128 + i]
    xt = np.ascontiguousarray(
        x.reshape(n, C, PW, 128).transpose(0, 3, 2, 1).reshape(n, 128, PW * C))
    consts = {"w1r": w1r, "w2r": w2r, "wpt": wpt, "b2c": b2c,
              "bpc": bpc, "expb1": expb1, "ident": ident, "identb": identb,
              "sel": sel, "selt": selt}
    in_maps = []
    for core in range(N_CORES):
        m = dict(consts)
        m["xs"] = np.ascontiguousarray(x[core * B:(core + 1) * B])
        m["xt"] = np.ascontiguousarray(xt[core * B:(core + 1) * B])
        in_maps.append(m)
    return in_maps


def run(inputs, trace=False):
    import os
    if not trace:
        # the axon NTFF profile hook is unavailable in this container; a
        # stray BASS_TRACE=1 in the environment would crash the run
        os.environ["BASS_NEVER_TRACE"] = "1"
    nc = _get_nc()
    in_maps = prep_inputs(**inputs)
    res = run_bass_kernel_spmd(nc, in_maps, list(range(N_CORES)), trace=trace)
    out = np.concatenate([res.results[i]["out"] for i in range(N_CORES)], axis=0)
    return out.astype(np.float32), res


def kernel(x, w1, b1, w2, b2, w_post, b_post):
    out, _ = run(dict(x=x, w1=w1, b1=b1, w2=w2, b2=b2,
                      w_post=w_post, b_post=b_post))
    return out


# revision 34
# speedup vs baseline: 1.0470x; 1.0004x over previous
"""ClusterAttention TRN2 kernel (v3).

Computation (per batch item):
  a  = conv1d_same(x, w1, b1)                      # (k, p) conv over p
  bm = conv1d_same(x^T, w2, b2)                    # (k, c) conv over c
  bm = bm @ x                                      # (k, p)
  probs = softmax(a + bm, axis=k)                  # (k, p)
  out = w_post @ (probs[:, None, :] * x[None]).reshape(k*c, p) + b_post

Sharding: data-parallel over batch n (32) across 8 cores, 4 per core.
All params replicated.

v3 design:
  - convs as dense matmuls with taps stacked on output partitions, then a
    log-tree of shifted adds on DVE (a-tree in bf16, bm-tree in f32)
  - logits computed DIRECTLY transposed: pa4T[p, k] per 128-wide p-window
    via selector matmuls (stationary = tree output D, moving = selT) plus
    x^T @ bmT matmuls; softmax runs on [p, k] tiles with no PE transposes
  - softmax normalization batched per half: one reciprocal + wide DVE
    multiplies (free-dim broadcast APs) instead of per-window ops
  - probs -> DRAM via XBAR dma transpose; broadcast back to [128, p] tiles
    with stride-0 partition-broadcast DMA reads (no gpsimd)
  - xT uploaded from host; bias adds on Act; consts loaded in few DMAs
"""
import sys

sys.path.insert(0, "/opt/trn_rl_repo")

import numpy as np

import concourse.bass as bass
import concourse.mybir as mybir
import concourse.tile as tile
from concourse import bacc
from concourse.bass_utils import run_bass_kernel_spmd

dt = mybir.dt

N_CORES = 8
B = 4            # batch items per core
C = 256          # channels
P = 2048         # sequence length
K = 16           # clusters
TAPS = 15
PAD = 7
CT = C // 128    # c-tiles
NCH = P // 512   # 512-wide p-chunks
PW = P // 128    # 128-wide p-windows
HPW = PW // 2    # p-windows per half
KG = 2           # clusters per broadcast group

AW = P + 16      # A_buf width: col j holds A[p = j - PAD], zeros outside
BMW = C + 16     # BM_buf width


def build_nc():
    nc = bacc.Bacc(None)

    xs_d = nc.dram_tensor("xs", [B, C, P], dt.float32, kind="ExternalInput")
    xt_d = nc.dram_tensor("xt", [B, 128, PW * C], dt.float32, kind="ExternalInput")
    w1r_d = nc.dram_tensor("w1r", [C, 256], dt.float32, kind="ExternalInput")
    w2r_d = nc.dram_tensor("w2r", [P, 256], dt.float32, kind="ExternalInput")
    wpt_d = nc.dram_tensor("wpt", [K * C, C], dt.bfloat16, kind="ExternalInput")
    b2_d = nc.dram_tensor("b2c", [K, 1], dt.float32, kind="ExternalInput")
    bp_d = nc.dram_tensor("bpc", [128, CT], dt.float32, kind="ExternalInput")
    eb1_d = nc.dram_tensor("expb1", [128, HPW * K], dt.float32, kind="ExternalInput")
    id_d = nc.dram_tensor("ident", [128, 128], dt.float32, kind="ExternalInput")
    sel_d = nc.dram_tensor("sel", [32, 32], dt.float32, kind="ExternalInput")
    selt_d = nc.dram_tensor("selt", [32, 32], dt.bfloat16, kind="ExternalInput")
    idb_d = nc.dram_tensor("identb", [128, 128], dt.bfloat16, kind="ExternalInput")
    out_d = nc.dram_tensor("out", [B, C, P], dt.float32, kind="ExternalOutput")

    F32R = dt.float32r
    BF16 = dt.bfloat16
    IDENT = mybir.ActivationFunctionType.Identity
    EXP = mybir.ActivationFunctionType.Exp

    with tile.TileContext(nc) as tc:
        with (
            tc.tile_pool(name="const", bufs=1) as cpool,
            tc.tile_pool(name="xs", bufs=2) as xpool,
            tc.tile_pool(name="xt", bufs=2) as xtpool,
            tc.tile_pool(name="abuf", bufs=1) as apool,
            tc.tile_pool(name="tree", bufs=2) as tpool,
            tc.tile_pool(name="sm", bufs=2) as smpool,
            tc.tile_pool(name="main", bufs=6) as mpool,
            tc.tile_pool(name="bc", bufs=6) as bcpool,
            tc.tile_pool(name="oc", bufs=2) as ocpool,
            tc.tile_pool(name="pd", bufs=2, space="DRAM") as pdpool,
            tc.tile_pool(name="pconv", bufs=2, space="PSUM") as pconv,
            tc.tile_pool(name="ptr", bufs=1, space="PSUM") as ptr,
            tc.tile_pool(name="pout", bufs=4, space="PSUM") as pout,
        ):
            # ---- constants (few, consolidated DMAs) ----
            w1rt = cpool.tile([128, 2, 256], F32R, tag="w1rt")
            nc.sync.dma_start(
                out=w1rt[:],
                in_=w1r_d.rearrange("(t i) c -> i t c", t=2).bitcast(F32R))
            ident = cpool.tile([128, 128], dt.float32, tag="ident")
            nc.sync.dma_start(out=ident[:], in_=id_d[:])
            sel = cpool.tile([32, 32], F32R, tag="sel")
            nc.sync.dma_start(out=sel[:], in_=sel_d[:].bitcast(F32R))
            selt = cpool.tile([32, 32], BF16, tag="selt")
            nc.sync.dma_start(out=selt[:], in_=selt_d[:])
            w2rt = cpool.tile([128, PW, 256], F32R, tag="w2rt")
            wptt = cpool.tile([128, 2 * K, 256], BF16, tag="wptt")
            b2c = cpool.tile([K, 1], dt.float32, tag="b2c")
            nc.sync.dma_start(out=b2c[:], in_=b2_d[:])
            bpc = cpool.tile([128, CT], dt.float32, tag="bpc")
            nc.sync.dma_start(out=bpc[:], in_=bp_d[:])
            expb1 = cpool.tile([128, HPW * K], dt.float32, tag="expb1")
            nc.sync.dma_start(out=expb1[:], in_=eb1_d[:])

            state = {}

            def emit_p1(b):
                """loads, xb convert, convs, trees, bm merge, bmT."""
                # ---- load x and xT ----
                xs = [xpool.tile([128, P], F32R, tag=f"xs{ct}", name=f"xs{ct}")
                      for ct in range(CT)]
                for ct in range(CT):
                    for hh in range(2):
                        nc.sync.dma_start(
                            out=xs[ct][:, hh * 1024:(hh + 1) * 1024],
                            in_=xs_d[b, ct * 128:(ct + 1) * 128,
                                     hh * 1024:(hh + 1) * 1024].bitcast(F32R))
                xT = xtpool.tile([128, PW * C], F32R, tag="xT", name="xT")
                for hh in range(4):
                    nc.sync.dma_start(
                        out=xT[:, hh * 1024:(hh + 1) * 1024],
                        in_=xt_d[b, :, hh * 1024:(hh + 1) * 1024].bitcast(F32R))
                if b == 0:
                    for hh in range(4):
                        nc.sync.dma_start(
                            out=w2rt[:, hh * 4:(hh + 1) * 4, :],
                            in_=w2r_d[hh * 512:(hh + 1) * 512]
                            .rearrange("(t i) c -> i t c", t=4).bitcast(F32R))

                # ---- bf16 copy of x for the y path ----
                xb = [xpool.tile([128, P], BF16, tag=f"xb{ct}",
                                 name=f"xb{ct}", bufs=3) for ct in range(CT)]
                for ct in range(CT):
                    nc.scalar.copy(xb[ct][:], xs[ct][:].bitcast(dt.float32))

                # ---- A conv matmuls: A[(t,kk), p] (bf16 store) ----
                A = [apool.tile([128, AW], BF16, tag=f"A{mc}", name=f"A{mc}")
                     for mc in range(2)]
                if b == 0:
                    # pads stay zero across batches (bufs=1 buffer reuse)
                    for mc in range(2):
                        nc.vector.memset(A[mc][:, 0:PAD], 0.0)
                        nc.vector.memset(A[mc][:, PAD + P:AW], 0.0)
                for mc in range(2):
                    for nch in range(NCH):
                        pa = pconv.tile([128, 512], dt.float32, tag="pconv")
                        for ct in range(CT):
                            nc.tensor.matmul(
                                pa[:],
                                w1rt[:, ct, mc * 128:(mc + 1) * 128],
                                xs[ct][:, nch * 512:(nch + 1) * 512],
                                start=(ct == 0), stop=(ct == CT - 1))
                        nc.scalar.copy(
                            A[mc][:, PAD + nch * 512:PAD + (nch + 1) * 512], pa[:])

                # ---- BM conv matmuls: BM[(t,kk), cc] (f32) ----
                BM = [apool.tile([128, BMW], dt.float32, tag=f"BM{mc}",
                                 name=f"BM{mc}") for mc in range(2)]
                if b == 0:
                    for mc in range(2):
                        nc.vector.memset(BM[mc][:, 0:PAD], 0.0)
                        nc.vector.memset(BM[mc][:, PAD + C:BMW], 0.0)
                for mc in range(2):
                    pb = pconv.tile([128, C], dt.float32, tag="pconv")
                    for pw in range(PW):
                        nc.tensor.matmul(
                            pb[:],
                            w2rt[:, pw, mc * 128:(mc + 1) * 128],
                            xT[:, pw * C:pw * C + C],
                            start=(pw == 0), stop=(pw == PW - 1))
                    nc.scalar.copy(BM[mc][:, PAD:PAD + C], pb[:])

                # ---- shift-add tree for a (bf16) ----
                HALF = P // 2
                Da = []
                for h in range(2):
                    o = h * HALF
                    w1_, w2_, w3_ = HALF + 8, HALF + 4, HALF + 2
                    Bl = tpool.tile([64, w1_], BF16, tag="tr", name="Bl")
                    Bh = tpool.tile([64, w1_], BF16, tag="tr", name="Bh")
                    nc.vector.tensor_add(Bl[:], A[0][0:64, o:o + w1_],
                                         A[1][0:64, o + 8:o + 8 + w1_])
                    nc.vector.tensor_add(Bh[:], A[0][64:128, o:o + w1_],
                                         A[1][64:128, o + 8:o + 8 + w1_])
                    Cl = tpool.tile([32, w2_], BF16, tag="trc", name="Cl")
                    Ch = tpool.tile([32, w2_], BF16, tag="trc", name="Ch")
                    nc.vector.tensor_add(Cl[:], Bl[0:32, 0:w2_],
                                         Bh[0:32, 4:4 + w2_])
                    nc.vector.tensor_add(Ch[:], Bl[32:64, 0:w2_],
                                         Bh[32:64, 4:4 + w2_])
                    D = tpool.tile([32, w3_], BF16, tag="trd", name="D", bufs=2)
                    nc.vector.tensor_add(D[:], Cl[0:32, 0:w3_],
                                         Ch[0:32, 2:2 + w3_])
                    Da.append(D)

                # ---- shift-add tree for bm: [16, C] (f32) ----
                bmt = smpool.tile([K, C], dt.float32, tag="bmt")
                w1_, w2_, w3_ = C + 8, C + 4, C + 2
                Bl = tpool.tile([64, w1_], dt.float32, tag="trb", name="Blb")
                Bh = tpool.tile([64, w1_], dt.float32, tag="trb", name="Bhb")
                nc.vector.tensor_add(Bl[:], BM[0][0:64, 0:w1_],
                                     BM[1][0:64, 8:8 + w1_])
                nc.vector.tensor_add(Bh[:], BM[0][64:128, 0:w1_],
                                     BM[1][64:128, 8:8 + w1_])
                Cl = tpool.tile([32, w2_], dt.float32, tag="trcb", name="Clb")
                Ch = tpool.tile([32, w2_], dt.float32, tag="trcb", name="Chb")
                nc.vector.tensor_add(Cl[:], Bl[0:32, 0:w2_], Bh[0:32, 4:4 + w2_])
                nc.vector.tensor_add(Ch[:], Bl[32:64, 0:w2_], Bh[32:64, 4:4 + w2_])
                Db = tpool.tile([32, w3_], F32R, tag="trdb", name="Db", bufs=2)
                nc.vector.tensor_add(Db[:], Cl[0:32, 0:w3_],
                                     Ch[0:32, 2:2 + w3_])
                pb4 = pconv.tile([K, C], dt.float32, tag="pconv", name="pb4")
                nc.tensor.matmul(pb4[:], sel[:, 0:16], Db[:, 0:C],
                                 start=True, stop=False)
                nc.tensor.matmul(pb4[:], sel[:, 16:32], Db[:, 1:1 + C],
                                 start=False, stop=True)
                nc.scalar.activation(bmt[:], pb4[:], IDENT, bias=b2c[:])

                # ---- bmT = bm^T [C, 16] (2 tiles) ----
                bmT = [smpool.tile([128, K], F32R, tag=f"bmT{ct}", name=f"bmT{ct}")
                       for ct in range(CT)]
                for ct in range(CT):
                    pt = pconv.tile([128, 512], dt.float32, tag="pconv", name="pt")
                    nc.tensor.transpose(
                        pt[0:128, 0:K], bmt[:, ct * 128:(ct + 1) * 128],
                        ident[0:K, 0:K])
                    nc.scalar.copy(bmT[ct][:], pt[0:128, 0:K])
                state[b] = dict(xs=xs, xb=xb, Da=Da, bmT=bmT)

            def emit_p2h(b, h):
                """transposed logits + softmax + probs -> DRAM for one half."""
                xs = state[b]["xs"]
                D = state[b]["Da"][h]
                bmT = state[b]["bmT"]
                if h == 0:
                    state[b]["probs_d"] = pdpool.tile(
                        [K, P], BF16, tag="probs_d", name="probs_d")
                probs_d = state[b]["probs_d"]
                # pa4T[p, k] = a-merge (selector mms on D) + x^T @ bmT
                expTh = smpool.tile([128, HPW * K], dt.float32,
                                    tag="expTh", name="expTh")
                esums = smpool.tile([128, HPW], dt.float32, tag="esums",
                                    name="esums")
                for pl in range(HPW):
                    pw = h * HPW + pl
                    o = pl * 128
                    pa4T = ptr.tile([128, K], dt.float32, tag="pa4T",
                                    bufs=2)
                    nc.tensor.matmul(pa4T[:], D[:, o:o + 128],
                                     selt[:, 0:16], start=True, stop=False)
                    nc.tensor.matmul(pa4T[:], D[:, o + 1:o + 129],
                                     selt[:, 16:32], start=False, stop=False)
                    for ct in range(CT):
                        nc.tensor.matmul(
                            pa4T[:],
                            xs[ct][:, pw * 128:(pw + 1) * 128],
                            bmT[ct][:],
                            start=False, stop=(ct == CT - 1))
                    negmx = smpool.tile([128, 1], dt.float32, tag="negmx")
                    nc.vector.tensor_reduce(
                        negmx[:], pa4T[:], axis=mybir.AxisListType.X,
                        op=mybir.AluOpType.max, negate=True)
                    nc.scalar.activation(
                        expTh[:, pl * K:(pl + 1) * K], pa4T[:],
                        EXP, bias=negmx[:])
                # esums/normalize batched over the half
                tmpE = smpool.tile([128, HPW * K], dt.float32, tag="tmpE",
                                   name="tmpE")
                nc.vector.tensor_mul(tmpE[:], expTh[:], expb1[:])
                nc.vector.tensor_reduce(
                    esums[:], tmpE[:].rearrange("p (a b) -> p a b", a=HPW),
                    axis=mybir.AxisListType.X, op=mybir.AluOpType.add)
                recips = smpool.tile([128, HPW], dt.float32, tag="recips",
                                     name="recips")
                nc.vector.reciprocal(recips[:], esums[:])
                pTh = smpool.tile([128, HPW * K], dt.float32, tag="probsTh",
                                  name="probsTh")
                nc.vector.tensor_mul(
                    pTh[:].rearrange("p (a b) -> p a b", a=HPW),
                    tmpE[:].rearrange("p (a b) -> p a b", a=HPW),
                    recips[:].unsqueeze(2).to_broadcast([128, HPW, K]))
                # probs -> DRAM: ppb[(a k), i] = pTh[i, a*K + k]; the DMA
                # scatters partition a*K+k to probs_d[k, h*1024 + a*128 + i]
                ppb = pconv.tile([128, 512], dt.float32, tag="pconv",
                                 name="ppb")
                nc.tensor.transpose(ppb[0:128, 0:128], pTh[:], ident[:])
                probsS = smpool.tile([128, 128], BF16, tag="probsS",
                                     name="probsS")
                nc.scalar.copy(probsS[:], ppb[0:128, 0:128])
                nc.scalar.dma_start(
                    out=probs_d[:, h * 1024:(h + 1) * 1024]
                    .rearrange("k (a i) -> a k i", a=HPW),
                    in_=probsS[:])
                bcts = []
                for kg in range(K // KG):
                    bct = bcpool.tile([128, KG, 1024], BF16,
                                      tag="bct", name="bct")
                    nc.sync.dma_start(
                        out=bct[:],
                        in_=probs_d[kg * KG:(kg + 1) * KG,
                                    h * 1024:(h + 1) * 1024]
                        .unsqueeze(0).partition_broadcast(128))
                    bcts.append(bct)
                state[b][f"bcts{h}"] = bcts
                if b == 0 and h == 0:
                    for hh in range(4):
                        nc.sync.dma_start(
                            out=wptt[:, hh * 8:(hh + 1) * 8, :],
                            in_=wpt_d[hh * 1024:(hh + 1) * 1024]
                            .rearrange("(t i) c -> i t c", t=8))

            def emit_p3h(b, half):
                """y build, main matmul, bias + store for one half."""
                xb = state[b]["xb"]
                chunks = [2 * half, 2 * half + 1]
                po = {}
                for nch in chunks:
                    for ot in range(CT):
                        po[(nch, ot)] = pout.tile(
                            [128, 512], dt.float32, tag="pout", name="po")
                for kg in range(K // KG):
                    bct = state[b][f"bcts{half}"][kg]
                    for j in range(KG):
                        kk = kg * KG + j
                        ys = []
                        for ct in range(CT):
                            y = mpool.tile([128, 1024], BF16,
                                           tag="y", name="y")
                            nc.vector.tensor_mul(
                                y[:],
                                xb[ct][:, half * 1024:(half + 1) * 1024],
                                bct[:, j, :])
                            ys.append(y)
                        for nch in chunks:
                            co = (nch % 2) * 512
                            for ct in range(CT):
                                for ot in range(CT):
                                    nc.tensor.matmul(
                                        po[(nch, ot)][:],
                                        wptt[:, kk * 2 + ct,
                                             ot * 128:(ot + 1) * 128],
                                        ys[ct][:, co:co + 512],
                                        start=(kg == 0 and j == 0 and ct == 0),
                                        stop=(kg == K // KG - 1 and j == KG - 1
                                              and ct == CT - 1))
                for ot in range(CT):
                    oc = ocpool.tile([128, 1024], dt.float32, tag="oc",
                                     name="oc")
                    for nch in chunks:
                        nc.scalar.activation(
                            oc[:, (nch % 2) * 512:(nch % 2) * 512 + 512],
                            po[(nch, ot)][:], IDENT,
                            bias=bpc[:, ot:ot + 1])
                    nc.scalar.dma_start(
                        out=out_d[b, ot * 128:(ot + 1) * 128,
                                  half * 1024:(half + 1) * 1024],
                        in_=oc[:])

            emit_p1(0)
            emit_p2h(0, 0)
            emit_p2h(0, 1)
            for b in range(1, B):
                emit_p1(b)
                emit_p3h(b - 1, 0)
                emit_p2h(b, 0)
                emit_p3h(b - 1, 1)
                emit_p2h(b, 1)
            emit_p3h(B - 1, 0)
            emit_p3h(B - 1, 1)

    nc.compile()
    return nc


_NC_CACHE = None


def _get_nc():
    global _NC_CACHE
    if _NC_CACHE is None:
        _NC_CACHE = build_nc()
    return _NC_CACHE


def prep_inputs(x, w1, b1, w2, b2, w_post, b_post):
    """Host-side rearrangement of weights; returns per-core in_maps."""
    import ml_dtypes
    x = np.asarray(x, dtype=np.float32)
    n = x.shape[0]
    w1r = np.ascontiguousarray(
        np.asarray(w1, np.float32).transpose(1, 2, 0).reshape(C, TAPS * K))
    w1r = np.concatenate([w1r, np.zeros((C, 256 - TAPS * K), np.float32)], axis=1)
    w2r = np.ascontiguousarray(
        np.asarray(w2, np.float32).transpose(1, 2, 0).reshape(P, TAPS * K))
    w2r = np.concatenate([w2r, np.zeros((P, 256 - TAPS * K), np.float32)], axis=1)
    wpt = np.ascontiguousarray(
        np.asarray(w_post, np.float32).T).astype(ml_dtypes.bfloat16)
    b2c = np.asarray(b2, np.float32).reshape(K, 1)
    bpc = np.ascontiguousarray(
        np.asarray(b_post, np.float32).reshape(CT, 128).T)
    # expb1[i, pl*K + k] = exp(b1[k]) for every partition i, half-window pl
    eb1 = np.exp(np.asarray(b1, np.float32))
    expb1 = np.ascontiguousarray(
        np.broadcast_to(np.tile(eb1, HPW)[None, :], (128, HPW * K)))
    ident = np.eye(128, dtype=np.float32)
    identb = ident.astype(ml_dtypes.bfloat16)
    sel = np.zeros((32, 32), np.float32)
    for i in range(16):
        sel[i, i] = 1.0
        sel[16 + i, 16 + i] = 1.0
    selt = sel.astype(ml_dtypes.bfloat16)
    # xt[b, i, pw*C + c] = x[b, c, pw*128 + i]
    xt = np.ascontiguousarray(
        x.reshape(n, C, PW, 128).transpose(0, 3, 2, 1).reshape(n, 128, PW * C))
    consts = {"w1r": w1r, "w2r": w2r, "wpt": wpt, "b2c": b2c,
              "bpc": bpc, "expb1": expb1, "ident": ident, "identb": identb,
              "sel": sel, "selt": selt}
    in_maps = []
    for core in range(N_CORES):
        m = dict(consts)
        m["xs"] = np.ascontiguousarray(x[core * B:(core + 1) * B])
        m["xt"] = np.ascontiguousarray(xt[core * B:(core + 1) * B])
        in_maps.append(m)
    return in_maps


def run(inputs, trace=False):
    import os
    if not trace:
        # the axon NTFF profile hook is unavailable in this container; a
        # stray BASS_TRACE=1 in the environment would crash the run
        os.environ["BASS_NEVER_TRACE"] = "1"
    nc = _get_nc()
    in_maps = prep_inputs(**inputs)
    res = run_bass_kernel_spmd(nc, in_maps, list(range(N_CORES)), trace=trace)
    out = np.concatenate([res.results[i]["out"] for i in range(N_CORES)], axis=0)
    return out.astype(np.float32), res


def kernel(x, w1, b1, w2, b2, w_post, b_post):
    out, _ = run(dict(x=x, w1=w1, b1=b1, w2=w2, b2=b2,
                      w_post=w_post, b_post=b_post))
    return out
